# revision 25
# baseline (speedup 1.0000x reference)
"""Conv1dFFT (truncated-spectrum FFT conv) on 8 trn2 cores — cost-model-shaped v2.

Math: reference computes out = irfft(trunc(rfft(xp)) * conj(trunc(rfft(wp))))[..., :W] + b
on a ring of size L. Equivalently out[n,f,t] = sum_c sum_s w[f,c,s] * P[n,c,t+s] + b[f]
with P[n,c,j] = sum_tau x[n,c,tau] * D(j - PAD - tau), D = Dirichlet kernel of (L, H).

Device decomposition of the Toeplitz D-matrix G[t, a] = D(a - PAD - t) (physical
coords, no parity games):
  - NEAR: column blocks of 128 (grid offset 64), each contracted against the two
    K=128 tau-chunks centered on the diagonal (covers d in ~(-196, 188) incl. the
    d=0 spike). Dense bf16 matmuls.
  - FAR: the off-band remainder is numerically rank ~64 (global randomized SVD on
    host). Contract T = x @ U once per row-tile (32 chunks x 64 cols), transpose T
    on DVE, then one V matmul per psum group accumulates the far field into the
    same psum as NEAR.
Stage 2 contracts the 9 filter taps as 4 K=128 matmuls + 1 K=64 matmul per
512-col window using a channel-duplicated one-column-shifted copy P2 of P.
Bias is fused into the PSUM->SBUF drain on the Activation engine; output fp16.

Sharding: pure data-parallel over batch N: 4 batch items per core; all cores
share identical constant tensors (D-near tiles, U/V far factors, weights).
"""

import numpy as np
import ml_dtypes

from contextlib import ExitStack

import concourse.bass as bass
import concourse.tile as tile
from concourse import bacc, mybir
from concourse.bass_utils import run_bass_kernel_spmd

# ---- problem constants (hardcoded; kernel.py must be self-contained) ----
N, C, W = 32, 64, 4096
F, WW = 128, 9
PAD = 4
OUT_W = W - WW + 1 + 2 * PAD                   # 4096
L = W + 2 * PAD + 2 * (WW - 1) + (OUT_W - 1)   # 8215
INIT_HALF = L // 2 + 1                         # 4108
IB = min(INIT_HALF - 1, int(INIT_HALF * 0.5) + 1)
HALF = INIT_HALF - IB                          # 2053
H = 2 * HALF - 1                               # 4105
J_TOT = W + 2 * PAD + 1                        # 4105 P columns needed

# ---- sharding / tiling ----
N_CORES = 8
NPC = N // N_CORES                             # 4 batch items per core
M_TILES = NPC // 2                             # 2 row-tiles of 128 (2 items x 64 ch)
KH = W // 128                                  # 32 K-chunks over taus
R_FAR = 64                                     # far-field rank
JC = 4112                                      # padded P width (>= J_TOT, /16)
WW2 = 5                                        # tap pairs (0,1)(2,3)(4,5)(6,7)(8,-)
NWIN = OUT_W // 512                            # 8 output windows per batch item

BF16 = ml_dtypes.bfloat16


def make_blocks():
    """Near column blocks: (a0, a1, k0, k1) — cols [a0,a1) vs tau chunks [k0,k1)."""
    blocks = []
    starts = [0] + list(range(64, J_TOT, 128))
    for a0 in starts:
        a1 = min(a0 + (64 if a0 == 0 else 128), J_TOT)
        c = (a0 + a1) / 2 - PAD
        best = None
        for k0 in range(int(c) // 128 - 2, int(c) // 128 + 2):
            t0, t1 = 128 * k0, 128 * (k0 + 2)
            lo, hi = a0 - PAD - (t1 - 1), (a1 - 1) - PAD - t0
            score = max(abs(lo), abs(hi))
            if best is None or score < best[0]:
                best = (score, k0)
        k0 = best[1]
        blocks.append((a0, a1, max(0, k0), min(KH, k0 + 2)))
    return blocks


BLOCKS = make_blocks()                         # 33 blocks
# psum groups: block 0 alone, then 8 groups of 4 blocks (last clipped to 457 wide)
GROUPS = [[0]] + [list(range(1 + 4 * g, 5 + 4 * g)) for g in range(8)]

_CONST_CACHE = {}


def _host_consts():
    """Dirichlet matrix G, near-block tiles, far-field factors U/V (all shared)."""
    if "c" in _CONST_CACHE:
        return _CONST_CACHE["c"]
    d = np.arange(-(W + PAD - 1), J_TOT - PAD, dtype=np.float64)
    with np.errstate(invalid="ignore", divide="ignore"):
        Dv = np.sin(np.pi * H * d / L) / (L * np.sin(np.pi * d / L))
    Dv[d == 0] = H / L

    taus = np.arange(W)
    cols = np.arange(J_TOT)
    G = Dv[(cols[None, :] - PAD - taus[:, None]) + (W + PAD - 1)]

    near_mask = np.zeros_like(G, dtype=bool)
    for (a0, a1, k0, k1) in BLOCKS:
        near_mask[128 * k0:128 * k1, a0:a1] = True
    Far = G * (~near_mask)

    rng = np.random.default_rng(0)
    Om = rng.normal(size=(J_TOT, 4 * R_FAR))
    Q, _ = np.linalg.qr(Far @ Om)
    u_, s_, vt_ = np.linalg.svd(Q.T @ Far, full_matrices=False)
    Ufar = Q @ u_[:, :R_FAR]                   # [W, R_FAR]
    Vfar = s_[:R_FAR, None] * vt_[:R_FAR, :]   # [R_FAR, J_TOT]

    # All interior blocks share one Toeplitz tile: a0 - 128*k0 == 64 for blocks
    # 1..31, so G[128*(k0+ci)+p, a0+col] = D(64 + col - 4 - 128*ci - p) is
    # block-independent. dn slots: 0 = block 0, 1 = interior, 2 = last block.
    dn = np.zeros((128, 3, 2, 128), np.float32)
    for blk, slot in ((0, 0), (1, 1), (len(BLOCKS) - 1, 2)):
        a0, a1, k0, k1 = BLOCKS[blk]
        for ci in range(k1 - k0):
            dn[:, slot, ci, 0:a1 - a0] = G[128 * (k0 + ci):128 * (k0 + ci + 1), a0:a1]
    for blk, (a0, a1, k0, k1) in enumerate(BLOCKS[1:-1], start=1):
        assert a0 - 128 * k0 == 64 and k1 - k0 == 2, (blk, BLOCKS[blk])
    dn = dn.astype(BF16)

    u = np.ascontiguousarray(
        Ufar.reshape(KH, 128, R_FAR).transpose(1, 0, 2)
    ).astype(BF16)                              # [128, KH, R_FAR]
    v = np.zeros((R_FAR, JC), np.float32)
    v[:, 0:J_TOT] = Vfar
    v = v.astype(BF16)

    _CONST_CACHE["c"] = (dn, u, v)
    return _CONST_CACHE["c"]


# constant blob layout [128, CBLOB] bf16: u | v(folded 2x) | w  (dn is its own
# small DMA so the first near matmuls start early)
U_OFF, V_OFF, W_OFF = 0, 2048, 4104
VFOLD = JC // 2                                 # 2056
CBLOB = 4744


def _pack_consts(w):
    dn, u, v = _host_consts()
    wc = w.transpose(1, 2, 0)                   # [c, s, f]
    wt = np.zeros((128, WW2, F), np.float32)
    for i in range(4):
        wt[0:64, i, :] = wc[:, 2 * i, :]
        wt[64:128, i, :] = wc[:, 2 * i + 1, :]
    wt[0:64, 4, :] = wc[:, 8, :]
    blob = np.zeros((128, CBLOB), BF16)
    blob[:, U_OFF:U_OFF + 2048] = u.reshape(128, 2048)
    blob[0:64, V_OFF:V_OFF + VFOLD] = v[:, 0:VFOLD]
    blob[64:128, V_OFF:V_OFF + VFOLD] = v[:, VFOLD:JC]
    blob[:, W_OFF:W_OFF + 640] = wt.reshape(128, 640).astype(BF16)
    return blob, dn.reshape(128, 768)


def build_nc():
    bf = mybir.dt.bfloat16
    f32 = mybir.dt.float32
    f16 = mybir.dt.float16
    nc = bacc.Bacc("TRN2", target_bir_lowering=False, debug=False)

    NB = len(BLOCKS)
    xeo_d = nc.dram_tensor("xeo", [M_TILES, 128, KH, 128], bf, kind="ExternalInput")
    dn_d = nc.dram_tensor("dn", [128, 768], bf, kind="ExternalInput")
    c_d = nc.dram_tensor("cblob", [128, CBLOB], bf, kind="ExternalInput")
    b_d = nc.dram_tensor("bias", [128, 1], f32, kind="ExternalInput")
    out_d = nc.dram_tensor("out", [NPC, F, OUT_W], f16, kind="ExternalOutput")

    with tile.TileContext(nc) as tc, ExitStack() as ctx:
        consts = ctx.enter_context(tc.tile_pool(name="consts", bufs=1))
        xpool = ctx.enter_context(tc.tile_pool(name="x", bufs=2))
        tpool = ctx.enter_context(tc.tile_pool(name="t", bufs=2))
        ppool = ctx.enter_context(tc.tile_pool(name="p", bufs=2))
        p2pool = ctx.enter_context(tc.tile_pool(name="p2", bufs=4))
        opool = ctx.enter_context(tc.tile_pool(name="o", bufs=8))
        # psum tiles are padded to 512 f32 cols (one full 2KB bank) so no two
        # accumulation groups ever share a bank (start=True clears whole-bank
        # has_written). 4 + 1 + 3 = 8 banks.
        ps_t = ctx.enter_context(tc.tile_pool(name="ps_t", bufs=1, space="PSUM"))
        ps_n = ctx.enter_context(tc.tile_pool(name="ps_n", bufs=4, space="PSUM"))
        ps_2 = ctx.enter_context(tc.tile_pool(name="ps_2", bufs=3, space="PSUM"))

        # ---- prologue: one consts blob + few large x pieces ----
        ctile = consts.tile([128, CBLOB], bf, name="c")
        dntile = consts.tile([128, 768], bf, name="dn")
        btile = consts.tile([128, 1], f32, name="b")
        xtiles = {}

        xt0 = xpool.tile([128, KH, 128], bf, name="xeo0", tag="xeo")
        xtiles[0] = xt0
        nc.scalar.dma_start(out=dntile[:], in_=dn_d[:])
        nc.sync.dma_start(out=xt0[:, 0:4, :], in_=xeo_d[0, :, 0:4, :])
        nc.scalar.dma_start(out=ctile[:, U_OFF:U_OFF + 2048], in_=c_d[:, U_OFF:U_OFF + 2048])
        nc.sync.dma_start(out=xt0[:, 4:18, :], in_=xeo_d[0, :, 4:18, :])
        nc.sync.dma_start(out=xt0[:, 18:32, :], in_=xeo_d[0, :, 18:32, :])
        nc.scalar.dma_start(out=ctile[:, V_OFF:CBLOB], in_=c_d[:, V_OFF:CBLOB])
        nc.scalar.dma_start(out=btile[:], in_=b_d[:])

        def load_x(m, eng):
            xt = xpool.tile([128, KH, 128], bf, name=f"xeo{m}", tag="xeo")
            for q in range(2):
                eng.dma_start(out=xt[:, 16 * q:16 * q + 16, :],
                              in_=xeo_d[m, :, 16 * q:16 * q + 16, :])
            xtiles[m] = xt

        # PE p-state warm-up: dependency-free matmuls on a zeroed scratch tile
        # keep the tensor engine "continuously busy" through the DMA-fed head,
        # so the first real matmuls already run at the full 2.4 GHz p-state.
        scr = consts.tile([128, 128], bf, name="scr")
        nc.vector.memset(scr[:, :], 0.0)
        for wi in range(22):
            pw = ps_2.tile([128, 512], mybir.dt.float32, name=f"warm{wi}", tag="ps2")
            nc.tensor.matmul(pw[:, 0:128], scr[:, :], scr[:, :],
                             start=True, stop=True)

        def near_group(m, g):
            xt = xtiles[m]
            c0 = BLOCKS[GROUPS[g][0]][0]
            c1 = BLOCKS[GROUPS[g][-1]][1]
            ps = ps_n.tile([128, 512], mybir.dt.float32, name=f"ps{m}_{g}", tag="psn")
            # start=True clears has_written for the whole PSUM bank, so only the
            # tile's first matmul may set it; per-element has_written then turns
            # each region's first write into an overwrite and the rest accumulate.
            first = True
            for blk in GROUPS[g]:
                a0, a1, k0, k1 = BLOCKS[blk]
                slot = 0 if blk == 0 else (2 if blk == len(BLOCKS) - 1 else 1)
                for ci in range(k1 - k0):
                    d0 = (slot * 2 + ci) * 128
                    nc.tensor.matmul(ps[:, a0 - c0:a1 - c0], xt[:, k0 + ci, :],
                                     dntile[:, d0:d0 + a1 - a0],
                                     start=first, stop=False)
                    first = False
            return ps, c0, c1

        def t_mms(m, psT, k0, k1):
            xt = xtiles[m]
            for k in range(k0, k1):
                u0 = U_OFF + k * R_FAR
                nc.tensor.matmul(psT[:, 0:R_FAR], xt[:, k, :],
                                 ctile[:, u0:u0 + R_FAR],
                                 start=(k == 0), stop=(k == KH - 1))

        def t_finish(m, psT, h):
            # tt duplicated on both partition halves: V matmuls for cols >= VFOLD
            # read the folded v from partitions 64:128 and operands must align.
            # h=0 feeds V of groups 0..4 (critical path); h=1 is emitted later.
            if h == 0:
                tcb = tpool.tile([128, R_FAR], bf, name=f"tc{m}", tag="tc")
                nc.vector.tensor_copy(out=tcb[:, :], in_=psT[:, 0:R_FAR])
                tt = tpool.tile([128, 128], bf, name=f"tt{m}", tag="tt")
                self_ = (tcb, tt)
            else:
                tcb, tt = psT
                self_ = (tcb, tt)
            for i in range(4):
                for j in range(R_FAR // 32):
                    nc.vector.transpose(
                        out=tt[64 * h + 32 * j:64 * h + 32 * j + 32,
                               32 * i:32 * i + 32],
                        in_=tcb[32 * i:32 * i + 32, 32 * j:32 * j + 32],
                    )
            return self_

        def v_mm(tt, ps, c0, c1):
            # folded v: cols < VFOLD on partitions 0:64, cols >= VFOLD on 64:128
            pieces = []
            if c0 < VFOLD:
                pieces.append((0, c0, min(c1, VFOLD)))
            if c1 > VFOLD:
                pieces.append((64, max(c0, VFOLD), c1))
            for pi, (h, p0, p1) in enumerate(pieces):
                v0 = V_OFF + p0 - (VFOLD if h else 0)
                nc.tensor.matmul(ps[:, p0 - c0:p1 - c0],
                                 tt[h:h + R_FAR, :],
                                 ctile[h:h + R_FAR, v0:v0 + p1 - p0],
                                 start=False, stop=(pi == len(pieces) - 1),
                                 tile_position=(h, 0))
            return

        # p2 copy column splits: half boundary at group 4 end (col 2112)
        CH = 64 + 512 * 4  # 2112

        def p2_half(m, ptile, p2s, half):
            if half == 0:
                un, sh = (0, CH), (0, CH - 1)
            else:
                un, sh = (CH, J_TOT), (CH - 1, J_TOT - 1)
            for uu in range(2):
                r0 = slice(64 * uu, 64 * uu + 64)
                nc.scalar.dma_start(out=p2s[uu][0:64, un[0]:un[1]],
                                    in_=ptile[r0, un[0]:un[1]])
                nc.scalar.dma_start(out=p2s[uu][64:128, sh[0]:sh[1]],
                                    in_=ptile[r0, sh[0] + 1:sh[1] + 1])

        def stage2_win(m, p2s, oas, uu, win):
            ps = ps_2.tile([128, 512], mybir.dt.float32,
                           name=f"ps2_{m}_{uu}_{win}", tag="ps2")
            j0 = 512 * win
            for i in range(4):
                w0 = W_OFF + i * 128
                nc.tensor.matmul(ps[:, :], ctile[:, w0:w0 + 128],
                                 p2s[uu][:, j0 + 2 * i:j0 + 2 * i + 512],
                                 start=(i == 0), stop=False)
            nc.tensor.matmul(ps[:, :], ctile[0:64, W_OFF + 512:W_OFF + 640],
                             p2s[uu][0:64, j0 + 8:j0 + 8 + 512],
                             start=False, stop=True)
            nc.scalar.activation(oas[uu][:, j0:j0 + 512], ps[:, :],
                                 mybir.ActivationFunctionType.Identity,
                                 bias=btile[:, 0:1])

        def out_flush(m, oas, uu, j0, j1):
            nc.sync.dma_start(out=out_d[2 * m + uu, :, j0:j1], in_=oas[uu][:, j0:j1])

        def do_m(m, carry):
            """Column-pipelined stage1 for one m-tile; emits carried stage-2
            window closures from the previous m into PE gaps. Returns this m's
            16 window closures (each also flushes out pieces when complete)."""
            psT = ps_t.tile([128, 512], mybir.dt.float32, name=f"psT{m}", tag="psT")
            ptile = ppool.tile([128, JC], bf, name=f"pt{m}", tag="pt")
            p2s = [p2pool.tile([128, JC], bf, name=f"p2_{m}_{uu}", tag="p2")
                   for uu in range(2)]
            oas = [opool.tile([128, OUT_W], mybir.dt.float16,
                              name=f"oa_{m}_{uu}", tag="o") for uu in range(2)]
            NG = len(GROUPS)
            pend = {}
            carry = list(carry)

            def emit_carry(n=1):
                for _ in range(n):
                    if carry:
                        carry.pop(0)()

            def win_closure(uu, win):
                def f():
                    stage2_win(m, p2s, oas, uu, win)
                    if win == 3:
                        out_flush(m, oas, uu, 0, 2048)
                    elif m == M_TILES - 1 and win >= 4:
                        # final m-tile: flush per 512-col window so the kernel
                        # tail only waits on one small out DMA
                        out_flush(m, oas, uu, 512 * win, 512 * win + 512)
                    elif win == 7:
                        out_flush(m, oas, uu, 2048, OUT_W)
                return f

            # keep at most 4 near groups open (4 ps_n banks); T interleaved as
            # u/x land; V+drain close each group before the next +4 opens
            pend[0] = near_group(m, 0)
            pend[1] = near_group(m, 1)
            emit_carry(1)
            pend[2] = near_group(m, 2)
            t_mms(m, psT, 0, 8)
            emit_carry(1)
            pend[3] = near_group(m, 3)
            t_mms(m, psT, 8, KH)
            tpair = t_finish(m, psT, 0)
            tt = tpair[1]
            for g in range(NG):
                if g == 3:
                    t_finish(m, tpair, 1)
                ps, c0, c1 = pend.pop(g)
                v_mm(tt, ps, c0, c1)
                nc.vector.tensor_copy(out=ptile[:, c0:c1], in_=ps[:, 0:c1 - c0])
                if g + 4 < NG:
                    pend[g + 4] = near_group(m, g + 4)
                emit_carry(1)
                if g == 4:
                    p2_half(m, ptile, p2s, 0)
                    # this m's first-half windows become available now; queue
                    # them behind carried windows from the previous m
                    carry += [win_closure(uu, w) for w in range(4) for uu in range(2)]
            p2_half(m, ptile, p2s, 1)
            emit_carry(len(carry))
            return [win_closure(uu, w) for w in range(4, 8) for uu in range(2)]

        carry = do_m(0, [])
        load_x(1, nc.sync)
        carry = do_m(1, carry)
        for f in carry:
            f()

    nc.compile()
    return nc


def _prep_inputs(x, w, b):
    blob, dn = _pack_consts(w)
    bias = np.ascontiguousarray(b.reshape(128, 1).astype(np.float32))

    in_maps = []
    for core in range(N_CORES):
        xh = x[core * NPC:(core + 1) * NPC].reshape(NPC * C, W)  # [256, 4096]
        # xeo[m, p, k, r] = xh[128*m + r, 128*k + p]
        xeo = np.ascontiguousarray(
            xh.T.reshape(KH, 128, M_TILES, 128).transpose(2, 1, 0, 3)
        ).astype(BF16)
        in_maps.append({"xeo": xeo, "dn": dn, "cblob": blob, "bias": bias})
    return in_maps


def run(x, w, b, trace=False):
    nc = build_nc()
    in_maps = _prep_inputs(x, w, b)
    res = run_bass_kernel_spmd(nc, in_maps, list(range(N_CORES)), trace=trace)
    out = np.empty((N, F, OUT_W), np.float32)
    for core in range(N_CORES):
        out[core * NPC:(core + 1) * NPC] = res.results[core]["out"].astype(np.float32)
    return out, res


def kernel(x, w, b):
    x = np.asarray(x, dtype=np.float32)
    w = np.asarray(w, dtype=np.float32)
    b = np.asarray(b, dtype=np.float32)
    out, _ = run(x, w, b, trace=False)
    return out


# revision 31
# speedup vs baseline: 1.0223x; 1.0223x over previous
"""Conv1dFFT (truncated-spectrum FFT conv) on 8 trn2 cores — cost-model-shaped v2.

Math: reference computes out = irfft(trunc(rfft(xp)) * conj(trunc(rfft(wp))))[..., :W] + b
on a ring of size L. Equivalently out[n,f,t] = sum_c sum_s w[f,c,s] * P[n,c,t+s] + b[f]
with P[n,c,j] = sum_tau x[n,c,tau] * D(j - PAD - tau), D = Dirichlet kernel of (L, H).

Device decomposition of the Toeplitz D-matrix G[t, a] = D(a - PAD - t) (physical
coords, no parity games):
  - NEAR: column blocks of 128 (grid offset 64), each contracted against the two
    K=128 tau-chunks centered on the diagonal (covers d in ~(-196, 188) incl. the
    d=0 spike). Dense bf16 matmuls.
  - FAR: the off-band remainder is numerically rank ~64 (global randomized SVD on
    host). Contract T = x @ U once per row-tile (32 chunks x 64 cols), transpose T
    on DVE, then one V matmul per psum group accumulates the far field into the
    same psum as NEAR.
Stage 2 contracts the 9 filter taps as 4 K=128 matmuls + 1 K=64 matmul per
512-col window using a channel-duplicated one-column-shifted copy P2 of P.
Bias is fused into the PSUM->SBUF drain on the Activation engine; output fp16.

Sharding: pure data-parallel over batch N: 4 batch items per core; all cores
share identical constant tensors (D-near tiles, U/V far factors, weights).
"""

import numpy as np
import ml_dtypes

from contextlib import ExitStack

import concourse.tile as tile
from concourse import bacc, mybir
from concourse.bass_utils import run_bass_kernel_spmd

# ---- problem constants (hardcoded; kernel.py must be self-contained) ----
N, C, W = 32, 64, 4096
F, WW = 128, 9
PAD = 4
OUT_W = W - WW + 1 + 2 * PAD                   # 4096
L = W + 2 * PAD + 2 * (WW - 1) + (OUT_W - 1)   # 8215
INIT_HALF = L // 2 + 1                         # 4108
IB = min(INIT_HALF - 1, int(INIT_HALF * 0.5) + 1)
HALF = INIT_HALF - IB                          # 2053
H = 2 * HALF - 1                               # 4105
J_TOT = W + 2 * PAD + 1                        # 4105 P columns needed

# ---- sharding / tiling ----
N_CORES = 8
NPC = N // N_CORES                             # 4 batch items per core
M_TILES = NPC // 2                             # 2 row-tiles of 128 (2 items x 64 ch)
KH = W // 128                                  # 32 K-chunks over taus
R_FAR = 64                                     # far-field rank
JC = 4112                                      # padded P width (>= J_TOT, /16)
WW2 = 5                                        # tap pairs (0,1)(2,3)(4,5)(6,7)(8,-)
NWIN = OUT_W // 512                            # 8 output windows per batch item

BF16 = ml_dtypes.bfloat16


def make_blocks():
    """Near column blocks: (a0, a1, k0, k1) — cols [a0,a1) vs tau chunks [k0,k1)."""
    blocks = []
    starts = [0] + list(range(64, J_TOT, 128))
    for a0 in starts:
        a1 = min(a0 + (64 if a0 == 0 else 128), J_TOT)
        c = (a0 + a1) / 2 - PAD
        best = None
        for k0 in range(int(c) // 128 - 2, int(c) // 128 + 2):
            t0, t1 = 128 * k0, 128 * (k0 + 2)
            lo, hi = a0 - PAD - (t1 - 1), (a1 - 1) - PAD - t0
            score = max(abs(lo), abs(hi))
            if best is None or score < best[0]:
                best = (score, k0)
        k0 = best[1]
        blocks.append((a0, a1, max(0, k0), min(KH, k0 + 2)))
    return blocks


BLOCKS = make_blocks()                         # 33 blocks
# psum groups: block 0 alone, then 8 groups of 4 blocks (last clipped to 457 wide)
GROUPS = [[0]] + [list(range(1 + 4 * g, 5 + 4 * g)) for g in range(8)]

_CONST_CACHE = {}


def _host_consts():
    """Dirichlet matrix G, near-block tiles, far-field factors U/V (all shared)."""
    if "c" in _CONST_CACHE:
        return _CONST_CACHE["c"]
    d = np.arange(-(W + PAD - 1), J_TOT - PAD, dtype=np.float64)
    with np.errstate(invalid="ignore", divide="ignore"):
        Dv = np.sin(np.pi * H * d / L) / (L * np.sin(np.pi * d / L))
    Dv[d == 0] = H / L

    taus = np.arange(W)
    cols = np.arange(J_TOT)
    G = Dv[(cols[None, :] - PAD - taus[:, None]) + (W + PAD - 1)]

    near_mask = np.zeros_like(G, dtype=bool)
    for (a0, a1, k0, k1) in BLOCKS:
        near_mask[128 * k0:128 * k1, a0:a1] = True
    Far = G * (~near_mask)

    rng = np.random.default_rng(0)
    Om = rng.normal(size=(J_TOT, 4 * R_FAR))
    Q, _ = np.linalg.qr(Far @ Om)
    u_, s_, vt_ = np.linalg.svd(Q.T @ Far, full_matrices=False)
    Ufar = Q @ u_[:, :R_FAR]                   # [W, R_FAR]
    Vfar = s_[:R_FAR, None] * vt_[:R_FAR, :]   # [R_FAR, J_TOT]

    # All interior blocks share one Toeplitz tile: a0 - 128*k0 == 64 for blocks
    # 1..31, so G[128*(k0+ci)+p, a0+col] = D(64 + col - 4 - 128*ci - p) is
    # block-independent. dn slots: 0 = block 0, 1 = interior, 2 = last block.
    dn = np.zeros((128, 3, 2, 128), np.float32)
    for blk, slot in ((0, 0), (1, 1), (len(BLOCKS) - 1, 2)):
        a0, a1, k0, k1 = BLOCKS[blk]
        for ci in range(k1 - k0):
            dn[:, slot, ci, 0:a1 - a0] = G[128 * (k0 + ci):128 * (k0 + ci + 1), a0:a1]
    for blk, (a0, a1, k0, k1) in enumerate(BLOCKS[1:-1], start=1):
        assert a0 - 128 * k0 == 64 and k1 - k0 == 2, (blk, BLOCKS[blk])
    dn = dn.astype(BF16)

    u = np.ascontiguousarray(
        Ufar.reshape(KH, 128, R_FAR).transpose(1, 0, 2)
    ).astype(BF16)                              # [128, KH, R_FAR]
    v = np.zeros((R_FAR, JC), np.float32)
    v[:, 0:J_TOT] = Vfar
    v = v.astype(BF16)

    _CONST_CACHE["c"] = (dn, u, v)
    return _CONST_CACHE["c"]


# constant blob layout [128, CBLOB] bf16: u | v(folded 2x) | w  (dn is its own
# small DMA so the first near matmuls start early)
U_OFF, V_OFF, W_OFF, I_OFF = 0, 2048, 4104, 4744
VFOLD = JC // 2                                 # 2056
CBLOB = 4872


def _pack_consts(w):
    dn, u, v = _host_consts()
    wc = w.transpose(1, 2, 0)                   # [c, s, f]
    wt = np.zeros((128, WW2, F), np.float32)
    for i in range(4):
        wt[0:64, i, :] = wc[:, 2 * i, :]
        wt[64:128, i, :] = wc[:, 2 * i + 1, :]
    wt[0:64, 4, :] = wc[:, 8, :]
    blob = np.zeros((128, CBLOB), BF16)
    blob[:, U_OFF:U_OFF + 2048] = u.reshape(128, 2048)
    blob[0:64, V_OFF:V_OFF + VFOLD] = v[:, 0:VFOLD]
    blob[64:128, V_OFF:V_OFF + VFOLD] = v[:, VFOLD:JC]
    blob[:, W_OFF:W_OFF + 640] = wt.reshape(128, 640).astype(BF16)
    blob[:, I_OFF:I_OFF + 128] = np.eye(128, dtype=BF16)
    return blob, dn.reshape(128, 768)


def build_nc():
    bf = mybir.dt.bfloat16
    f32 = mybir.dt.float32
    f16 = mybir.dt.float16
    nc = bacc.Bacc("TRN2", target_bir_lowering=False, debug=False)

    xeo_d = nc.dram_tensor("xeo", [M_TILES, 128, KH, 128], bf, kind="ExternalInput")
    dn_d = nc.dram_tensor("dn", [128, 768], bf, kind="ExternalInput")
    c_d = nc.dram_tensor("cblob", [128, CBLOB], bf, kind="ExternalInput")
    b_d = nc.dram_tensor("bias", [128, 1], f32, kind="ExternalInput")
    out_d = nc.dram_tensor("out", [NPC, F, OUT_W], f16, kind="ExternalOutput")

    with tile.TileContext(nc) as tc, ExitStack() as ctx:
        consts = ctx.enter_context(tc.tile_pool(name="consts", bufs=1))
        xpool = ctx.enter_context(tc.tile_pool(name="x", bufs=2))
        tpool = ctx.enter_context(tc.tile_pool(name="t", bufs=2))
        ppool = ctx.enter_context(tc.tile_pool(name="p", bufs=2))
        p2pool = ctx.enter_context(tc.tile_pool(name="p2", bufs=4))
        opool = ctx.enter_context(tc.tile_pool(name="o", bufs=8))
        # psum tiles are padded to 512 f32 cols (one full 2KB bank) so no two
        # accumulation groups ever share a bank (start=True clears whole-bank
        # has_written). 4 + 1 + 3 = 8 banks.
        ps_t = ctx.enter_context(tc.tile_pool(name="ps_t", bufs=1, space="PSUM"))
        ps_n = ctx.enter_context(tc.tile_pool(name="ps_n", bufs=4, space="PSUM"))
        ps_2 = ctx.enter_context(tc.tile_pool(name="ps_2", bufs=3, space="PSUM"))

        # ---- prologue: one consts blob + few large x pieces ----
        ctile = consts.tile([128, CBLOB], bf, name="c")
        dntile = consts.tile([128, 768], bf, name="dn")
        btile = consts.tile([128, 1], f32, name="b")
        xtiles = {}

        xt0 = xpool.tile([128, KH, 128], bf, name="xeo0", tag="xeo")
        xtiles[0] = xt0
        nc.scalar.dma_start(out=dntile[:], in_=dn_d[:])
        nc.sync.dma_start(out=xt0[:, 0:4, :], in_=xeo_d[0, :, 0:4, :])
        nc.scalar.dma_start(out=ctile[:, U_OFF:U_OFF + 2048], in_=c_d[:, U_OFF:U_OFF + 2048])
        nc.sync.dma_start(out=xt0[:, 4:18, :], in_=xeo_d[0, :, 4:18, :])
        nc.sync.dma_start(out=xt0[:, 18:32, :], in_=xeo_d[0, :, 18:32, :])
        nc.scalar.dma_start(out=ctile[:, V_OFF:CBLOB], in_=c_d[:, V_OFF:CBLOB])
        nc.scalar.dma_start(out=btile[:], in_=b_d[:])

        def load_x(m, eng):
            xt = xpool.tile([128, KH, 128], bf, name=f"xeo{m}", tag="xeo")
            for q in range(2):
                eng.dma_start(out=xt[:, 16 * q:16 * q + 16, :],
                              in_=xeo_d[m, :, 16 * q:16 * q + 16, :])
            xtiles[m] = xt

        # PE p-state warm-up: dependency-free matmuls on a zeroed scratch tile
        # keep the tensor engine "continuously busy" through the DMA-fed head,
        # so the first real matmuls already run at the full 2.4 GHz p-state.
        scr = consts.tile([128, 128], bf, name="scr")
        nc.vector.memset(scr[:, :], 0.0)
        for wi in range(32):
            pw = ps_2.tile([128, 512], mybir.dt.float32, name=f"warm{wi}", tag="ps2")
            nc.tensor.matmul(pw[:, 0:128], scr[:, :], scr[:, :],
                             start=True, stop=True)

        def near_group(m, g):
            xt = xtiles[m]
            c0 = BLOCKS[GROUPS[g][0]][0]
            c1 = BLOCKS[GROUPS[g][-1]][1]
            ps = ps_n.tile([128, 512], mybir.dt.float32, name=f"ps{m}_{g}", tag="psn")
            # start=True clears has_written for the whole PSUM bank, so only the
            # tile's first matmul may set it; per-element has_written then turns
            # each region's first write into an overwrite and the rest accumulate.
            first = True
            for blk in GROUPS[g]:
                a0, a1, k0, k1 = BLOCKS[blk]
                slot = 0 if blk == 0 else (2 if blk == len(BLOCKS) - 1 else 1)
                for ci in range(k1 - k0):
                    d0 = (slot * 2 + ci) * 128
                    nc.tensor.matmul(ps[:, a0 - c0:a1 - c0], xt[:, k0 + ci, :],
                                     dntile[:, d0:d0 + a1 - a0],
                                     start=first, stop=False)
                    first = False
            return ps, c0, c1

        def t_mms(m, psT, k0, k1):
            xt = xtiles[m]
            for k in range(k0, k1):
                u0 = U_OFF + k * R_FAR
                nc.tensor.matmul(psT[:, 0:R_FAR], xt[:, k, :],
                                 ctile[:, u0:u0 + R_FAR],
                                 start=(k == 0), stop=(k == KH - 1))

        def t_finish(m, psT, h):
            # transpose T on the PE (53 ns) instead of a chain of DVE stream
            # transposes; tt is duplicated on both partition halves because V
            # matmuls for cols >= VFOLD read folded v from partitions 64:128.
            # h=0 feeds V of groups 0..4 (critical path); h=1 is emitted later.
            if h == 0:
                tcb = tpool.tile([128, R_FAR], bf, name=f"tc{m}", tag="tc")
                nc.vector.tensor_copy(out=tcb[:, :], in_=psT[:, 0:R_FAR])
                ptp = ps_t.tile([128, 1024], bf, name=f"ptp{m}", tag="psT")
                nc.tensor.transpose(ptp[0:R_FAR, 0:128], tcb[:, :],
                                    ctile[:, I_OFF:I_OFF + 128])
                tt = tpool.tile([128, 128], bf, name=f"tt{m}", tag="tt")
                nc.vector.tensor_copy(out=tt[0:R_FAR, :], in_=ptp[0:R_FAR, 0:128])
                self_ = (ptp, tt)
            else:
                ptp, tt = psT
                nc.vector.tensor_copy(out=tt[64:64 + R_FAR, :],
                                      in_=ptp[0:R_FAR, 0:128])
            return (ptp, tt)

        def v_mm(tt, ps, c0, c1):
            # folded v: cols < VFOLD on partitions 0:64, cols >= VFOLD on 64:128
            pieces = []
            if c0 < VFOLD:
                pieces.append((0, c0, min(c1, VFOLD)))
            if c1 > VFOLD:
                pieces.append((64, max(c0, VFOLD), c1))
            for pi, (h, p0, p1) in enumerate(pieces):
                v0 = V_OFF + p0 - (VFOLD if h else 0)
                nc.tensor.matmul(ps[:, p0 - c0:p1 - c0],
                                 tt[h:h + R_FAR, :],
                                 ctile[h:h + R_FAR, v0:v0 + p1 - p0],
                                 start=False, stop=(pi == len(pieces) - 1),
                                 tile_position=(h, 0))
            return

        # p2 copy column splits: half boundary at group 4 end (col 2112)
        CH = 64 + 512 * 4  # 2112

        def p2_half(m, ptile, p2s, half):
            if half == 0:
                un, sh = (0, CH), (0, CH - 1)
            else:
                un, sh = (CH, J_TOT), (CH - 1, J_TOT - 1)
            for uu in range(2):
                r0 = slice(64 * uu, 64 * uu + 64)
                nc.scalar.dma_start(out=p2s[uu][0:64, un[0]:un[1]],
                                    in_=ptile[r0, un[0]:un[1]])
                nc.scalar.dma_start(out=p2s[uu][64:128, sh[0]:sh[1]],
                                    in_=ptile[r0, sh[0] + 1:sh[1] + 1])

        def stage2_win(m, p2s, oas, uu, win):
            ps = ps_2.tile([128, 512], mybir.dt.float32,
                           name=f"ps2_{m}_{uu}_{win}", tag="ps2")
            j0 = 512 * win
            for i in range(4):
                w0 = W_OFF + i * 128
                nc.tensor.matmul(ps[:, :], ctile[:, w0:w0 + 128],
                                 p2s[uu][:, j0 + 2 * i:j0 + 2 * i + 512],
                                 start=(i == 0), stop=False)
            nc.tensor.matmul(ps[:, :], ctile[0:64, W_OFF + 512:W_OFF + 640],
                             p2s[uu][0:64, j0 + 8:j0 + 8 + 512],
                             start=False, stop=True)
            nc.scalar.activation(oas[uu][:, j0:j0 + 512], ps[:, :],
                                 mybir.ActivationFunctionType.Identity,
                                 bias=btile[:, 0:1])

        def out_flush(m, oas, uu, j0, j1):
            nc.sync.dma_start(out=out_d[2 * m + uu, :, j0:j1], in_=oas[uu][:, j0:j1])

        def do_m(m, carry):
            """Column-pipelined stage1 for one m-tile; emits carried stage-2
            window closures from the previous m into PE gaps. Returns this m's
            16 window closures (each also flushes out pieces when complete)."""
            psT = ps_t.tile([128, 512], mybir.dt.float32, name=f"psT{m}", tag="psT")
            ptile = ppool.tile([128, JC], bf, name=f"pt{m}", tag="pt")
            p2s = [p2pool.tile([128, JC], bf, name=f"p2_{m}_{uu}", tag="p2")
                   for uu in range(2)]
            oas = [opool.tile([128, OUT_W], mybir.dt.float16,
                              name=f"oa_{m}_{uu}", tag="o") for uu in range(2)]
            NG = len(GROUPS)
            pend = {}
            carry = list(carry)

            def emit_carry(n=1):
                for _ in range(n):
                    if carry:
                        carry.pop(0)()

            def win_closure(uu, win):
                def f():
                    stage2_win(m, p2s, oas, uu, win)
                    if win == 3:
                        out_flush(m, oas, uu, 0, 2048)
                    elif m == M_TILES - 1 and win >= 4:
                        # final m-tile: flush per 512-col window so the kernel
                        # tail only waits on one small out DMA
                        out_flush(m, oas, uu, 512 * win, 512 * win + 512)
                    elif win == 7:
                        out_flush(m, oas, uu, 2048, OUT_W)
                return f

            # keep at most 4 near groups open (4 ps_n banks); T interleaved as
            # u/x land; V+drain close each group before the next +4 opens
            pend[0] = near_group(m, 0)
            pend[1] = near_group(m, 1)
            emit_carry(1)
            pend[2] = near_group(m, 2)
            t_mms(m, psT, 0, 8)
            emit_carry(1)
            pend[3] = near_group(m, 3)
            t_mms(m, psT, 8, KH)
            tpair = t_finish(m, psT, 0)
            tt = tpair[1]
            for g in range(NG):
                if g == 3:
                    t_finish(m, tpair, 1)
                ps, c0, c1 = pend.pop(g)
                v_mm(tt, ps, c0, c1)
                nc.vector.tensor_copy(out=ptile[:, c0:c1], in_=ps[:, 0:c1 - c0])
                if g + 4 < NG:
                    pend[g + 4] = near_group(m, g + 4)
                emit_carry(1)
                if g == 4:
                    p2_half(m, ptile, p2s, 0)
                    # this m's first-half windows become available now; queue
                    # them behind carried windows from the previous m
                    carry += [win_closure(uu, w) for w in range(4) for uu in range(2)]
            p2_half(m, ptile, p2s, 1)
            emit_carry(len(carry))
            return [win_closure(uu, w) for w in range(4, 8) for uu in range(2)]

        carry = do_m(0, [])
        load_x(1, nc.sync)
        carry = do_m(1, carry)
        for f in carry:
            f()

    nc.compile()
    return nc


def _prep_inputs(x, w, b):
    blob, dn = _pack_consts(w)
    bias = np.ascontiguousarray(b.reshape(128, 1).astype(np.float32))

    in_maps = []
    for core in range(N_CORES):
        xh = x[core * NPC:(core + 1) * NPC].reshape(NPC * C, W)  # [256, 4096]
        # xeo[m, p, k, r] = xh[128*m + r, 128*k + p]
        xeo = np.ascontiguousarray(
            xh.T.reshape(KH, 128, M_TILES, 128).transpose(2, 1, 0, 3)
        ).astype(BF16)
        in_maps.append({"xeo": xeo, "dn": dn, "cblob": blob, "bias": bias})
    return in_maps


def run(x, w, b, trace=False):
    nc = build_nc()
    in_maps = _prep_inputs(x, w, b)
    try:
        res = run_bass_kernel_spmd(nc, in_maps, list(range(N_CORES)), trace=trace)
    except Exception:
        # transient NRT device errors on first execution after a fresh NEFF
        # load have been observed; one retry has always succeeded
        res = run_bass_kernel_spmd(nc, in_maps, list(range(N_CORES)), trace=trace)
    out = np.empty((N, F, OUT_W), np.float32)
    for core in range(N_CORES):
        out[core * NPC:(core + 1) * NPC] = res.results[core]["out"].astype(np.float32)
    return out, res


def kernel(x, w, b):
    x = np.asarray(x, dtype=np.float32)
    w = np.asarray(w, dtype=np.float32)
    b = np.asarray(b, dtype=np.float32)
    out, _ = run(x, w, b, trace=False)
    return out


# revision 40
# speedup vs baseline: 1.0264x; 1.0040x over previous
"""Conv1dFFT (truncated-spectrum FFT conv) on 8 trn2 cores — cost-model-shaped v2.

Math: reference computes out = irfft(trunc(rfft(xp)) * conj(trunc(rfft(wp))))[..., :W] + b
on a ring of size L. Equivalently out[n,f,t] = sum_c sum_s w[f,c,s] * P[n,c,t+s] + b[f]
with P[n,c,j] = sum_tau x[n,c,tau] * D(j - PAD - tau), D = Dirichlet kernel of (L, H).

Device decomposition of the Toeplitz D-matrix G[t, a] = D(a - PAD - t) (physical
coords, no parity games):
  - NEAR: column blocks of 128 (grid offset 64), each contracted against the two
    K=128 tau-chunks centered on the diagonal (covers d in ~(-196, 188) incl. the
    d=0 spike). Dense bf16 matmuls.
  - FAR: the off-band remainder is numerically rank ~64 (global randomized SVD on
    host). Contract T = x @ U once per row-tile (32 chunks x 64 cols), transpose T
    on DVE, then one V matmul per psum group accumulates the far field into the
    same psum as NEAR.
Stage 2 contracts the 9 filter taps as 4 K=128 matmuls + 1 K=64 matmul per
512-col window using a channel-duplicated one-column-shifted copy P2 of P.
Bias is fused into the PSUM->SBUF drain on the Activation engine; output fp16.

Sharding: pure data-parallel over batch N: 4 batch items per core; all cores
share identical constant tensors (D-near tiles, U/V far factors, weights).
"""

import numpy as np
import ml_dtypes

from contextlib import ExitStack

import concourse.tile as tile
from concourse import bacc, mybir
from concourse.bass_utils import run_bass_kernel_spmd

# ---- problem constants (hardcoded; kernel.py must be self-contained) ----
N, C, W = 32, 64, 4096
F, WW = 128, 9
PAD = 4
OUT_W = W - WW + 1 + 2 * PAD                   # 4096
L = W + 2 * PAD + 2 * (WW - 1) + (OUT_W - 1)   # 8215
INIT_HALF = L // 2 + 1                         # 4108
IB = min(INIT_HALF - 1, int(INIT_HALF * 0.5) + 1)
HALF = INIT_HALF - IB                          # 2053
H = 2 * HALF - 1                               # 4105
J_TOT = W + 2 * PAD + 1                        # 4105 P columns needed

# ---- sharding / tiling ----
N_CORES = 8
NPC = N // N_CORES                             # 4 batch items per core
M_TILES = NPC // 2                             # 2 row-tiles of 128 (2 items x 64 ch)
KH = W // 128                                  # 32 K-chunks over taus
R_FAR = 64                                     # far-field rank
JC = 4112                                      # padded P width (>= J_TOT, /16)
WW2 = 5                                        # tap pairs (0,1)(2,3)(4,5)(6,7)(8,-)
NWIN = OUT_W // 512                            # 8 output windows per batch item

BF16 = ml_dtypes.bfloat16


def make_blocks():
    """Near column blocks: (a0, a1, k0, k1) — cols [a0,a1) vs tau chunks [k0,k1)."""
    blocks = []
    starts = [0] + list(range(64, J_TOT, 128))
    for a0 in starts:
        a1 = min(a0 + (64 if a0 == 0 else 128), J_TOT)
        c = (a0 + a1) / 2 - PAD
        best = None
        for k0 in range(int(c) // 128 - 2, int(c) // 128 + 2):
            t0, t1 = 128 * k0, 128 * (k0 + 2)
            lo, hi = a0 - PAD - (t1 - 1), (a1 - 1) - PAD - t0
            score = max(abs(lo), abs(hi))
            if best is None or score < best[0]:
                best = (score, k0)
        k0 = best[1]
        blocks.append((a0, a1, max(0, k0), min(KH, k0 + 2)))
    return blocks


BLOCKS = make_blocks()                         # 33 blocks
# psum groups: block 0 alone, then 8 groups of 4 blocks (last clipped to 457 wide)
GROUPS = [[0]] + [list(range(1 + 4 * g, 5 + 4 * g)) for g in range(8)]

_CONST_CACHE = {}


def _host_consts():
    """Dirichlet matrix G, near-block tiles, far-field factors U/V (all shared)."""
    if "c" in _CONST_CACHE:
        return _CONST_CACHE["c"]
    d = np.arange(-(W + PAD - 1), J_TOT - PAD, dtype=np.float64)
    with np.errstate(invalid="ignore", divide="ignore"):
        Dv = np.sin(np.pi * H * d / L) / (L * np.sin(np.pi * d / L))
    Dv[d == 0] = H / L

    taus = np.arange(W)
    cols = np.arange(J_TOT)
    G = Dv[(cols[None, :] - PAD - taus[:, None]) + (W + PAD - 1)]

    near_mask = np.zeros_like(G, dtype=bool)
    for (a0, a1, k0, k1) in BLOCKS:
        near_mask[128 * k0:128 * k1, a0:a1] = True
    Far = G * (~near_mask)

    rng = np.random.default_rng(0)
    Om = rng.normal(size=(J_TOT, 4 * R_FAR))
    Q, _ = np.linalg.qr(Far @ Om)
    u_, s_, vt_ = np.linalg.svd(Q.T @ Far, full_matrices=False)
    Ufar = Q @ u_[:, :R_FAR]                   # [W, R_FAR]
    Vfar = s_[:R_FAR, None] * vt_[:R_FAR, :]   # [R_FAR, J_TOT]

    # All interior blocks share one Toeplitz tile: a0 - 128*k0 == 64 for blocks
    # 1..31, so G[128*(k0+ci)+p, a0+col] = D(64 + col - 4 - 128*ci - p) is
    # block-independent. dn slots: 0 = block 0, 1 = interior, 2 = last block.
    dn = np.zeros((128, 3, 2, 128), np.float32)
    for blk, slot in ((0, 0), (1, 1), (len(BLOCKS) - 1, 2)):
        a0, a1, k0, k1 = BLOCKS[blk]
        for ci in range(k1 - k0):
            dn[:, slot, ci, 0:a1 - a0] = G[128 * (k0 + ci):128 * (k0 + ci + 1), a0:a1]
    for blk, (a0, a1, k0, k1) in enumerate(BLOCKS[1:-1], start=1):
        assert a0 - 128 * k0 == 64 and k1 - k0 == 2, (blk, BLOCKS[blk])
    dn = dn.astype(BF16)

    u = np.ascontiguousarray(
        Ufar.reshape(KH, 128, R_FAR).transpose(1, 0, 2)
    ).astype(BF16)                              # [128, KH, R_FAR]
    v = np.zeros((R_FAR, JC), np.float32)
    v[:, 0:J_TOT] = Vfar
    v = v.astype(BF16)

    _CONST_CACHE["c"] = (dn, u, v)
    return _CONST_CACHE["c"]


# constant blob layout [128, CBLOB] bf16: u | v(folded 2x) | w  (dn is its own
# small DMA so the first near matmuls start early)
U_OFF, V_OFF, W_OFF, I_OFF = 0, 2048, 4104, 4744
VFOLD = JC // 2                                 # 2056
CBLOB = 4872


def _pack_consts(w):
    dn, u, v = _host_consts()
    wc = w.transpose(1, 2, 0)                   # [c, s, f]
    wt = np.zeros((128, WW2, F), np.float32)
    for i in range(4):
        wt[0:64, i, :] = wc[:, 2 * i, :]
        wt[64:128, i, :] = wc[:, 2 * i + 1, :]
    wt[0:64, 4, :] = wc[:, 8, :]
    blob = np.zeros((128, CBLOB), BF16)
    blob[:, U_OFF:U_OFF + 2048] = u.reshape(128, 2048)
    blob[0:64, V_OFF:V_OFF + VFOLD] = v[:, 0:VFOLD]
    blob[64:128, V_OFF:V_OFF + VFOLD] = v[:, VFOLD:JC]
    blob[:, W_OFF:W_OFF + 640] = wt.reshape(128, 640).astype(BF16)
    blob[:, I_OFF:I_OFF + 128] = np.eye(128, dtype=BF16)
    return blob, dn.reshape(128, 768)


def build_nc():
    bf = mybir.dt.bfloat16
    f32 = mybir.dt.float32
    f16 = mybir.dt.float16
    nc = bacc.Bacc("TRN2", target_bir_lowering=False, debug=False)

    xeo_d = nc.dram_tensor("xeo", [M_TILES, 128, KH, 128], bf, kind="ExternalInput")
    dn_d = nc.dram_tensor("dn", [128, 768], bf, kind="ExternalInput")
    c_d = nc.dram_tensor("cblob", [128, CBLOB], bf, kind="ExternalInput")
    b_d = nc.dram_tensor("bias", [128, 1], f32, kind="ExternalInput")
    out_d = nc.dram_tensor("out", [NPC, F, OUT_W], f16, kind="ExternalOutput")

    with tile.TileContext(nc) as tc, ExitStack() as ctx:
        consts = ctx.enter_context(tc.tile_pool(name="consts", bufs=1))
        xpool = ctx.enter_context(tc.tile_pool(name="x", bufs=2))
        tpool = ctx.enter_context(tc.tile_pool(name="t", bufs=2))
        ppool = ctx.enter_context(tc.tile_pool(name="p", bufs=2))
        p2pool = ctx.enter_context(tc.tile_pool(name="p2", bufs=4))
        opool = ctx.enter_context(tc.tile_pool(name="o", bufs=8))
        # psum tiles are padded to 512 f32 cols (one full 2KB bank) so no two
        # accumulation groups ever share a bank (start=True clears whole-bank
        # has_written). 4 + 1 + 3 = 8 banks.
        ps_t = ctx.enter_context(tc.tile_pool(name="ps_t", bufs=1, space="PSUM"))
        ps_n = ctx.enter_context(tc.tile_pool(name="ps_n", bufs=5, space="PSUM"))
        ps_2 = ctx.enter_context(tc.tile_pool(name="ps_2", bufs=2, space="PSUM"))

        # ---- prologue: one consts blob + few large x pieces ----
        ctile = consts.tile([128, CBLOB], bf, name="c")
        dntile = consts.tile([128, 768], bf, name="dn")
        btile = consts.tile([128, 1], f32, name="b")
        xtiles = {}

        xt0 = xpool.tile([128, KH, 128], bf, name="xeo0", tag="xeo")
        xtiles[0] = xt0
        nc.scalar.dma_start(out=dntile[:], in_=dn_d[:])
        nc.sync.dma_start(out=xt0[:, 0:8, :], in_=xeo_d[0, :, 0:8, :])
        nc.sync.dma_start(out=xt0[:, 8:20, :], in_=xeo_d[0, :, 8:20, :])
        nc.sync.dma_start(out=ctile[:, U_OFF:U_OFF + 2048], in_=c_d[:, U_OFF:U_OFF + 2048])
        nc.sync.dma_start(out=xt0[:, 20:32, :], in_=xeo_d[0, :, 20:32, :])
        nc.scalar.dma_start(out=ctile[:, V_OFF:CBLOB], in_=c_d[:, V_OFF:CBLOB])
        nc.scalar.dma_start(out=btile[:], in_=b_d[:])

        def load_x(m, eng):
            xt = xpool.tile([128, KH, 128], bf, name=f"xeo{m}", tag="xeo")
            for q in range(2):
                eng.dma_start(out=xt[:, 16 * q:16 * q + 16, :],
                              in_=xeo_d[m, :, 16 * q:16 * q + 16, :])
            xtiles[m] = xt

        # PE p-state warm-up: dependency-free matmuls on a zeroed scratch tile
        # keep the tensor engine "continuously busy" through the DMA-fed head,
        # so the first real matmuls already run at the full 2.4 GHz p-state.
        scr = consts.tile([128, 128], bf, name="scr")
        nc.vector.memset(scr[:, :], 0.0)
        for wi in range(32):
            pw = ps_2.tile([128, 512], mybir.dt.float32, name=f"warm{wi}", tag="ps2")
            nc.tensor.matmul(pw[:, 0:128], scr[:, :], scr[:, :],
                             start=True, stop=True)

        def near_group(m, g):
            xt = xtiles[m]
            c0 = BLOCKS[GROUPS[g][0]][0]
            c1 = BLOCKS[GROUPS[g][-1]][1]
            ps = ps_n.tile([128, 512], mybir.dt.float32, name=f"ps{m}_{g}", tag="psn")
            # start=True clears has_written for the whole PSUM bank, so only the
            # tile's first matmul may set it; per-element has_written then turns
            # each region's first write into an overwrite and the rest accumulate.
            first = True
            for blk in GROUPS[g]:
                a0, a1, k0, k1 = BLOCKS[blk]
                slot = 0 if blk == 0 else (2 if blk == len(BLOCKS) - 1 else 1)
                for ci in range(k1 - k0):
                    d0 = (slot * 2 + ci) * 128
                    nc.tensor.matmul(ps[:, a0 - c0:a1 - c0], xt[:, k0 + ci, :],
                                     dntile[:, d0:d0 + a1 - a0],
                                     start=first, stop=False)
                    first = False
            return ps, c0, c1

        def t_mms(m, psT, k0, k1):
            xt = xtiles[m]
            for k in range(k0, k1):
                u0 = U_OFF + k * R_FAR
                nc.tensor.matmul(psT[:, 0:R_FAR], xt[:, k, :],
                                 ctile[:, u0:u0 + R_FAR],
                                 start=(k == 0), stop=(k == KH - 1))

        def t_finish(m, psT, h):
            # transpose T on the PE (53 ns) instead of a chain of DVE stream
            # transposes; tt is duplicated on both partition halves because V
            # matmuls for cols >= VFOLD read folded v from partitions 64:128.
            # h=0 feeds V of groups 0..4 (critical path); h=1 is emitted later.
            if h == 0:
                tcb = tpool.tile([128, R_FAR], bf, name=f"tc{m}", tag="tc")
                nc.vector.tensor_copy(out=tcb[:, :], in_=psT[:, 0:R_FAR])
                ptp = ps_t.tile([128, 1024], bf, name=f"ptp{m}", tag="psT")
                nc.tensor.transpose(ptp[0:R_FAR, 0:128], tcb[:, :],
                                    ctile[:, I_OFF:I_OFF + 128])
                tt = tpool.tile([128, 128], bf, name=f"tt{m}", tag="tt")
                nc.vector.tensor_copy(out=tt[0:R_FAR, :], in_=ptp[0:R_FAR, 0:128])
                self_ = (ptp, tt)
            else:
                ptp, tt = psT
                nc.vector.tensor_copy(out=tt[64:64 + R_FAR, :],
                                      in_=ptp[0:R_FAR, 0:128])
            return (ptp, tt)

        def v_mm(tt, ps, c0, c1):
            # folded v: cols < VFOLD on partitions 0:64, cols >= VFOLD on 64:128
            pieces = []
            if c0 < VFOLD:
                pieces.append((0, c0, min(c1, VFOLD)))
            if c1 > VFOLD:
                pieces.append((64, max(c0, VFOLD), c1))
            for pi, (h, p0, p1) in enumerate(pieces):
                v0 = V_OFF + p0 - (VFOLD if h else 0)
                nc.tensor.matmul(ps[:, p0 - c0:p1 - c0],
                                 tt[h:h + R_FAR, :],
                                 ctile[h:h + R_FAR, v0:v0 + p1 - p0],
                                 start=False, stop=(pi == len(pieces) - 1),
                                 tile_position=(h, 0))
            return

        # p2 copy column splits: half boundary at group 4 end (col 2112)
        CH = 64 + 512 * 4  # 2112

        def p2_half(m, ptile, p2s, half):
            if half == 0:
                un, sh = (0, CH), (0, CH - 1)
            else:
                un, sh = (CH, J_TOT), (CH - 1, J_TOT - 1)
            for uu in range(2):
                r0 = slice(64 * uu, 64 * uu + 64)
                nc.scalar.dma_start(out=p2s[uu][0:64, un[0]:un[1]],
                                    in_=ptile[r0, un[0]:un[1]])
                nc.scalar.dma_start(out=p2s[uu][64:128, sh[0]:sh[1]],
                                    in_=ptile[r0, sh[0] + 1:sh[1] + 1])

        def stage2_win(m, p2s, oas, uu, win, j0=None, wid=512):
            ps = ps_2.tile([128, 512], mybir.dt.float32,
                           name=f"ps2_{m}_{uu}_{win}_{j0}", tag="ps2")
            if j0 is None:
                j0 = 512 * win
            for i in range(4):
                w0 = W_OFF + i * 128
                nc.tensor.matmul(ps[:, 0:wid], ctile[:, w0:w0 + 128],
                                 p2s[uu][:, j0 + 2 * i:j0 + 2 * i + wid],
                                 start=(i == 0), stop=False)
            nc.tensor.matmul(ps[:, 0:wid], ctile[0:64, W_OFF + 512:W_OFF + 640],
                             p2s[uu][0:64, j0 + 8:j0 + 8 + wid],
                             start=False, stop=True)
            nc.scalar.activation(oas[uu][:, j0:j0 + wid], ps[:, 0:wid],
                                 mybir.ActivationFunctionType.Identity,
                                 bias=btile[:, 0:1])

        def out_flush(m, oas, uu, j0, j1):
            nc.sync.dma_start(out=out_d[2 * m + uu, :, j0:j1], in_=oas[uu][:, j0:j1])

        def do_m(m, carry):
            """Column-pipelined stage1 for one m-tile; emits carried stage-2
            window closures from the previous m into PE gaps. Returns this m's
            16 window closures (each also flushes out pieces when complete)."""
            psT = ps_t.tile([128, 512], mybir.dt.float32, name=f"psT{m}", tag="psT")
            ptile = ppool.tile([128, JC], bf, name=f"pt{m}", tag="pt")
            p2s = [p2pool.tile([128, JC], bf, name=f"p2_{m}_{uu}", tag="p2")
                   for uu in range(2)]
            oas = [opool.tile([128, OUT_W], mybir.dt.float16,
                              name=f"oa_{m}_{uu}", tag="o") for uu in range(2)]
            NG = len(GROUPS)
            pend = {}
            carry = list(carry)

            def emit_carry(n=1):
                for _ in range(n):
                    if carry:
                        carry.pop(0)()

            def win_closure(uu, win):
                def f():
                    stage2_win(m, p2s, oas, uu, win)
                    if win == 3:
                        out_flush(m, oas, uu, 0, 2048)
                    elif m == M_TILES - 1 and win >= 4:
                        # final m-tile: flush per 512-col window so the kernel
                        # tail only waits on one small out DMA
                        out_flush(m, oas, uu, 512 * win, 512 * win + 512)
                    elif win == 7:
                        out_flush(m, oas, uu, 2048, OUT_W)
                return f

            # 5 near groups open (5 ps_n banks) while x[0:20] lands, then the
            # whole T chain (u + x tail); V+drain close each group before the
            # next +5 opens
            pend[0] = near_group(m, 0)
            pend[1] = near_group(m, 1)
            emit_carry(1)
            pend[2] = near_group(m, 2)
            emit_carry(1)
            pend[3] = near_group(m, 3)
            emit_carry(1)
            pend[4] = near_group(m, 4)
            t_mms(m, psT, 0, 20)
            emit_carry(1)
            t_mms(m, psT, 20, KH)
            tpair = t_finish(m, psT, 0)
            tt = tpair[1]
            for g in range(NG):
                if g == 3:
                    t_finish(m, tpair, 1)
                ps, c0, c1 = pend.pop(g)
                v_mm(tt, ps, c0, c1)
                nc.vector.tensor_copy(out=ptile[:, c0:c1], in_=ps[:, 0:c1 - c0])
                if g + 5 < NG:
                    pend[g + 5] = near_group(m, g + 5)
                emit_carry(1)
                if g == 4:
                    p2_half(m, ptile, p2s, 0)
                    # this m's first-half windows become available now; queue
                    # them behind carried windows from the previous m
                    carry += [win_closure(uu, w) for w in range(4) for uu in range(2)]
            p2_half(m, ptile, p2s, 1)
            emit_carry(len(carry))
            return [win_closure(uu, w) for w in range(4, 8) for uu in range(2)]

        carry = do_m(0, [])
        load_x(1, nc.sync)
        carry = do_m(1, carry)
        for f in carry:
            f()

    nc.compile()
    return nc


def _prep_inputs(x, w, b):
    blob, dn = _pack_consts(w)
    bias = np.ascontiguousarray(b.reshape(128, 1).astype(np.float32))

    in_maps = []
    for core in range(N_CORES):
        xh = x[core * NPC:(core + 1) * NPC].reshape(NPC * C, W)  # [256, 4096]
        # xeo[m, p, k, r] = xh[128*m + r, 128*k + p]
        xeo = np.ascontiguousarray(
            xh.T.reshape(KH, 128, M_TILES, 128).transpose(2, 1, 0, 3)
        ).astype(BF16)
        in_maps.append({"xeo": xeo, "dn": dn, "cblob": blob, "bias": bias})
    return in_maps


def run(x, w, b, trace=False):
    nc = build_nc()
    in_maps = _prep_inputs(x, w, b)
    try:
        res = run_bass_kernel_spmd(nc, in_maps, list(range(N_CORES)), trace=trace)
    except Exception:
        # transient NRT device errors on first execution after a fresh NEFF
        # load have been observed; one retry has always succeeded
        res = run_bass_kernel_spmd(nc, in_maps, list(range(N_CORES)), trace=trace)
    out = np.empty((N, F, OUT_W), np.float32)
    for core in range(N_CORES):
        out[core * NPC:(core + 1) * NPC] = res.results[core]["out"].astype(np.float32)
    return out, res


def kernel(x, w, b):
    x = np.asarray(x, dtype=np.float32)
    w = np.asarray(w, dtype=np.float32)
    b = np.asarray(b, dtype=np.float32)
    out, _ = run(x, w, b, trace=False)
    return out


# revision 44
# speedup vs baseline: 1.0344x; 1.0077x over previous
"""Conv1dFFT (truncated-spectrum FFT conv) on 8 trn2 cores — cost-model-shaped v2.

Math: reference computes out = irfft(trunc(rfft(xp)) * conj(trunc(rfft(wp))))[..., :W] + b
on a ring of size L. Equivalently out[n,f,t] = sum_c sum_s w[f,c,s] * P[n,c,t+s] + b[f]
with P[n,c,j] = sum_tau x[n,c,tau] * D(j - PAD - tau), D = Dirichlet kernel of (L, H).

Device decomposition of the Toeplitz D-matrix G[t, a] = D(a - PAD - t) (physical
coords, no parity games):
  - NEAR: column blocks of 128 (grid offset 64), each contracted against the two
    K=128 tau-chunks centered on the diagonal (covers d in ~(-196, 188) incl. the
    d=0 spike). Dense bf16 matmuls.
  - FAR: the off-band remainder is numerically rank ~64 (global randomized SVD on
    host). Contract T = x @ U once per row-tile (32 chunks x 64 cols), transpose T
    on DVE, then one V matmul per psum group accumulates the far field into the
    same psum as NEAR.
Stage 2 contracts the 9 filter taps as 4 K=128 matmuls + 1 K=64 matmul per
512-col window using a channel-duplicated one-column-shifted copy P2 of P.
Bias is fused into the PSUM->SBUF drain on the Activation engine; output fp16.

Sharding: pure data-parallel over batch N: 4 batch items per core; all cores
share identical constant tensors (D-near tiles, U/V far factors, weights).
"""

import numpy as np
import ml_dtypes

from contextlib import ExitStack

import concourse.tile as tile
from concourse import bacc, mybir
from concourse.bass_utils import run_bass_kernel_spmd

# ---- problem constants (hardcoded; kernel.py must be self-contained) ----
N, C, W = 32, 64, 4096
F, WW = 128, 9
PAD = 4
OUT_W = W - WW + 1 + 2 * PAD                   # 4096
L = W + 2 * PAD + 2 * (WW - 1) + (OUT_W - 1)   # 8215
INIT_HALF = L // 2 + 1                         # 4108
IB = min(INIT_HALF - 1, int(INIT_HALF * 0.5) + 1)
HALF = INIT_HALF - IB                          # 2053
H = 2 * HALF - 1                               # 4105
J_TOT = W + 2 * PAD + 1                        # 4105 P columns needed

# ---- sharding / tiling ----
N_CORES = 8
NPC = N // N_CORES                             # 4 batch items per core
M_TILES = NPC // 2                             # 2 row-tiles of 128 (2 items x 64 ch)
KH = W // 128                                  # 32 K-chunks over taus
R_FAR = 48                                     # far-field rank
JC = 4112                                      # padded P width (>= J_TOT, /16)
WW2 = 5                                        # tap pairs (0,1)(2,3)(4,5)(6,7)(8,-)
NWIN = OUT_W // 512                            # 8 output windows per batch item

BF16 = ml_dtypes.bfloat16


def make_blocks():
    """Near column blocks: (a0, a1, k0, k1) — cols [a0,a1) vs tau chunks [k0,k1)."""
    blocks = []
    starts = [0] + list(range(64, J_TOT, 128))
    for a0 in starts:
        a1 = min(a0 + (64 if a0 == 0 else 128), J_TOT)
        c = (a0 + a1) / 2 - PAD
        best = None
        for k0 in range(int(c) // 128 - 2, int(c) // 128 + 2):
            t0, t1 = 128 * k0, 128 * (k0 + 2)
            lo, hi = a0 - PAD - (t1 - 1), (a1 - 1) - PAD - t0
            score = max(abs(lo), abs(hi))
            if best is None or score < best[0]:
                best = (score, k0)
        k0 = best[1]
        blocks.append((a0, a1, max(0, k0), min(KH, k0 + 2)))
    return blocks


BLOCKS = make_blocks()                         # 33 blocks
# psum groups: block 0 alone, then 8 groups of 4 blocks (last clipped to 457 wide)
GROUPS = [[0]] + [list(range(1 + 4 * g, 5 + 4 * g)) for g in range(8)]

_CONST_CACHE = {}


def _host_consts():
    """Dirichlet matrix G, near-block tiles, far-field factors U/V (all shared)."""
    if "c" in _CONST_CACHE:
        return _CONST_CACHE["c"]
    d = np.arange(-(W + PAD - 1), J_TOT - PAD, dtype=np.float64)
    with np.errstate(invalid="ignore", divide="ignore"):
        Dv = np.sin(np.pi * H * d / L) / (L * np.sin(np.pi * d / L))
    Dv[d == 0] = H / L

    taus = np.arange(W)
    cols = np.arange(J_TOT)
    G = Dv[(cols[None, :] - PAD - taus[:, None]) + (W + PAD - 1)]

    near_mask = np.zeros_like(G, dtype=bool)
    for (a0, a1, k0, k1) in BLOCKS:
        near_mask[128 * k0:128 * k1, a0:a1] = True
    Far = G * (~near_mask)

    rng = np.random.default_rng(0)
    Om = rng.normal(size=(J_TOT, 4 * R_FAR))
    Q, _ = np.linalg.qr(Far @ Om)
    u_, s_, vt_ = np.linalg.svd(Q.T @ Far, full_matrices=False)
    Ufar = Q @ u_[:, :R_FAR]                   # [W, R_FAR]
    Vfar = s_[:R_FAR, None] * vt_[:R_FAR, :]   # [R_FAR, J_TOT]

    # All interior blocks share one Toeplitz tile: a0 - 128*k0 == 64 for blocks
    # 1..31, so G[128*(k0+ci)+p, a0+col] = D(64 + col - 4 - 128*ci - p) is
    # block-independent. dn slots: 0 = block 0, 1 = interior, 2 = last block.
    dn = np.zeros((128, 3, 2, 128), np.float32)
    for blk, slot in ((0, 0), (1, 1), (len(BLOCKS) - 1, 2)):
        a0, a1, k0, k1 = BLOCKS[blk]
        for ci in range(k1 - k0):
            dn[:, slot, ci, 0:a1 - a0] = G[128 * (k0 + ci):128 * (k0 + ci + 1), a0:a1]
    for blk, (a0, a1, k0, k1) in enumerate(BLOCKS[1:-1], start=1):
        assert a0 - 128 * k0 == 64 and k1 - k0 == 2, (blk, BLOCKS[blk])
    dn = dn.astype(BF16)

    u = np.ascontiguousarray(
        Ufar.reshape(KH, 128, R_FAR).transpose(1, 0, 2)
    ).astype(BF16)                              # [128, KH, R_FAR]
    v = np.zeros((R_FAR, JC), np.float32)
    v[:, 0:J_TOT] = Vfar
    v = v.astype(BF16)

    _CONST_CACHE["c"] = (dn, u, v)
    return _CONST_CACHE["c"]


# constant blob layout [128, CBLOB] bf16: u | v(folded 2x) | w  (dn is its own
# small DMA so the first near matmuls start early)
U_OFF, V_OFF, W_OFF, I_OFF = 0, 1536, 3592, 4232
VFOLD = JC // 2                                 # 2056
CBLOB = 4360


def _pack_consts(w):
    dn, u, v = _host_consts()
    wc = w.transpose(1, 2, 0)                   # [c, s, f]
    wt = np.zeros((128, WW2, F), np.float32)
    for i in range(4):
        wt[0:64, i, :] = wc[:, 2 * i, :]
        wt[64:128, i, :] = wc[:, 2 * i + 1, :]
    wt[0:64, 4, :] = wc[:, 8, :]
    blob = np.zeros((128, CBLOB), BF16)
    blob[:, U_OFF:U_OFF + KH * R_FAR] = u.reshape(128, KH * R_FAR)
    blob[0:R_FAR, V_OFF:V_OFF + VFOLD] = v[:, 0:VFOLD]
    blob[64:64 + R_FAR, V_OFF:V_OFF + VFOLD] = v[:, VFOLD:JC]
    blob[:, W_OFF:W_OFF + 640] = wt.reshape(128, 640).astype(BF16)
    blob[:, I_OFF:I_OFF + 128] = np.eye(128, dtype=BF16)
    return blob, dn.reshape(128, 768)


def build_nc():
    bf = mybir.dt.bfloat16
    f32 = mybir.dt.float32
    f16 = mybir.dt.float16
    nc = bacc.Bacc("TRN2", target_bir_lowering=False, debug=False)

    xeo_d = nc.dram_tensor("xeo", [M_TILES, 128, KH, 128], bf, kind="ExternalInput")
    dn_d = nc.dram_tensor("dn", [128, 768], bf, kind="ExternalInput")
    c_d = nc.dram_tensor("cblob", [128, CBLOB], bf, kind="ExternalInput")
    b_d = nc.dram_tensor("bias", [128, 1], f32, kind="ExternalInput")
    out_d = nc.dram_tensor("out", [NPC, F, OUT_W], f16, kind="ExternalOutput")

    with tile.TileContext(nc) as tc, ExitStack() as ctx:
        consts = ctx.enter_context(tc.tile_pool(name="consts", bufs=1))
        xpool = ctx.enter_context(tc.tile_pool(name="x", bufs=2))
        tpool = ctx.enter_context(tc.tile_pool(name="t", bufs=2))
        ppool = ctx.enter_context(tc.tile_pool(name="p", bufs=2))
        p2pool = ctx.enter_context(tc.tile_pool(name="p2", bufs=4))
        opool = ctx.enter_context(tc.tile_pool(name="o", bufs=8))
        # psum tiles are padded to 512 f32 cols (one full 2KB bank) so no two
        # accumulation groups ever share a bank (start=True clears whole-bank
        # has_written). 4 + 1 + 3 = 8 banks.
        ps_t = ctx.enter_context(tc.tile_pool(name="ps_t", bufs=1, space="PSUM"))
        ps_n = ctx.enter_context(tc.tile_pool(name="ps_n", bufs=5, space="PSUM"))
        ps_2 = ctx.enter_context(tc.tile_pool(name="ps_2", bufs=2, space="PSUM"))

        # ---- prologue: one consts blob + few large x pieces ----
        ctile = consts.tile([128, CBLOB], bf, name="c")
        dntile = consts.tile([128, 768], bf, name="dn")
        btile = consts.tile([128, 1], f32, name="b")
        xtiles = {}

        xt0 = xpool.tile([128, KH, 128], bf, name="xeo0", tag="xeo")
        xtiles[0] = xt0
        nc.scalar.dma_start(out=dntile[:], in_=dn_d[:])
        nc.sync.dma_start(out=xt0[:, 0:8, :], in_=xeo_d[0, :, 0:8, :])
        nc.sync.dma_start(out=xt0[:, 8:20, :], in_=xeo_d[0, :, 8:20, :])
        nc.sync.dma_start(out=ctile[:, U_OFF:U_OFF + KH * R_FAR], in_=c_d[:, U_OFF:U_OFF + KH * R_FAR])
        nc.sync.dma_start(out=xt0[:, 20:32, :], in_=xeo_d[0, :, 20:32, :])
        nc.scalar.dma_start(out=ctile[:, V_OFF:CBLOB], in_=c_d[:, V_OFF:CBLOB])
        nc.scalar.dma_start(out=btile[:], in_=b_d[:])

        def load_x(m, eng):
            xt = xpool.tile([128, KH, 128], bf, name=f"xeo{m}", tag="xeo")
            for q in range(2):
                eng.dma_start(out=xt[:, 16 * q:16 * q + 16, :],
                              in_=xeo_d[m, :, 16 * q:16 * q + 16, :])
            xtiles[m] = xt

        # PE p-state warm-up: dependency-free matmuls on a zeroed scratch tile
        # keep the tensor engine "continuously busy" through the DMA-fed head,
        # so the first real matmuls already run at the full 2.4 GHz p-state.
        scr = consts.tile([128, 128], bf, name="scr")
        nc.vector.memset(scr[:, :], 0.0)
        for wi in range(32):
            pw = ps_2.tile([128, 512], mybir.dt.float32, name=f"warm{wi}", tag="ps2")
            nc.tensor.matmul(pw[:, 0:128], scr[:, :], scr[:, :],
                             start=True, stop=True)

        def near_group(m, g):
            xt = xtiles[m]
            c0 = BLOCKS[GROUPS[g][0]][0]
            c1 = BLOCKS[GROUPS[g][-1]][1]
            ps = ps_n.tile([128, 512], mybir.dt.float32, name=f"ps{m}_{g}", tag="psn")
            # start=True clears has_written for the whole PSUM bank, so only the
            # tile's first matmul may set it; per-element has_written then turns
            # each region's first write into an overwrite and the rest accumulate.
            first = True
            for blk in GROUPS[g]:
                a0, a1, k0, k1 = BLOCKS[blk]
                slot = 0 if blk == 0 else (2 if blk == len(BLOCKS) - 1 else 1)
                for ci in range(k1 - k0):
                    d0 = (slot * 2 + ci) * 128
                    nc.tensor.matmul(ps[:, a0 - c0:a1 - c0], xt[:, k0 + ci, :],
                                     dntile[:, d0:d0 + a1 - a0],
                                     start=first, stop=False)
                    first = False
            return ps, c0, c1

        def t_mms(m, psT, k0, k1):
            xt = xtiles[m]
            for k in range(k0, k1):
                u0 = U_OFF + k * R_FAR
                nc.tensor.matmul(psT[:, 0:R_FAR], xt[:, k, :],
                                 ctile[:, u0:u0 + R_FAR],
                                 start=(k == 0), stop=(k == KH - 1))

        def t_finish(m, psT, h):
            # transpose T on the PE (53 ns) instead of a chain of DVE stream
            # transposes; tt is duplicated on both partition halves because V
            # matmuls for cols >= VFOLD read folded v from partitions 64:128.
            # h=0 feeds V of groups 0..4 (critical path); h=1 is emitted later.
            if h == 0:
                tcb = tpool.tile([128, R_FAR], bf, name=f"tc{m}", tag="tc")
                nc.vector.tensor_copy(out=tcb[:, :], in_=psT[:, 0:R_FAR])
                ptp = ps_t.tile([128, 1024], bf, name=f"ptp{m}", tag="psT")
                nc.tensor.transpose(ptp[0:R_FAR, 0:128], tcb[:, :],
                                    ctile[:, I_OFF:I_OFF + 128])
                tt = tpool.tile([128, 128], bf, name=f"tt{m}", tag="tt")
                nc.vector.tensor_copy(out=tt[0:R_FAR, :], in_=ptp[0:R_FAR, 0:128])
                self_ = (ptp, tt)
            else:
                ptp, tt = psT
                nc.vector.tensor_copy(out=tt[64:64 + R_FAR, :],
                                      in_=ptp[0:R_FAR, 0:128])
            return (ptp, tt)

        def v_mm(tt, ps, c0, c1):
            # folded v: cols < VFOLD on partitions 0:64, cols >= VFOLD on 64:128
            pieces = []
            if c0 < VFOLD:
                pieces.append((0, c0, min(c1, VFOLD)))
            if c1 > VFOLD:
                pieces.append((64, max(c0, VFOLD), c1))
            for pi, (h, p0, p1) in enumerate(pieces):
                v0 = V_OFF + p0 - (VFOLD if h else 0)
                nc.tensor.matmul(ps[:, p0 - c0:p1 - c0],
                                 tt[h:h + R_FAR, :],
                                 ctile[h:h + R_FAR, v0:v0 + p1 - p0],
                                 start=False, stop=(pi == len(pieces) - 1),
                                 tile_position=(h, 0))
            return

        # p2 copy column splits: half boundary at group 4 end (col 2112)
        CH = 64 + 512 * 4  # 2112

        def p2_half(m, ptile, p2s, half):
            if half == 0:
                un, sh = (0, CH), (0, CH - 1)
            else:
                un, sh = (CH, J_TOT), (CH - 1, J_TOT - 1)
            for uu in range(2):
                r0 = slice(64 * uu, 64 * uu + 64)
                nc.scalar.dma_start(out=p2s[uu][0:64, un[0]:un[1]],
                                    in_=ptile[r0, un[0]:un[1]])
                nc.scalar.dma_start(out=p2s[uu][64:128, sh[0]:sh[1]],
                                    in_=ptile[r0, sh[0] + 1:sh[1] + 1])

        def stage2_win(m, p2s, oas, uu, win, j0=None, wid=512):
            ps = ps_2.tile([128, 512], mybir.dt.float32,
                           name=f"ps2_{m}_{uu}_{win}_{j0}", tag="ps2")
            if j0 is None:
                j0 = 512 * win
            for i in range(4):
                w0 = W_OFF + i * 128
                nc.tensor.matmul(ps[:, 0:wid], ctile[:, w0:w0 + 128],
                                 p2s[uu][:, j0 + 2 * i:j0 + 2 * i + wid],
                                 start=(i == 0), stop=False)
            nc.tensor.matmul(ps[:, 0:wid], ctile[0:64, W_OFF + 512:W_OFF + 640],
                             p2s[uu][0:64, j0 + 8:j0 + 8 + wid],
                             start=False, stop=True)
            nc.scalar.activation(oas[uu][:, j0:j0 + wid], ps[:, 0:wid],
                                 mybir.ActivationFunctionType.Identity,
                                 bias=btile[:, 0:1])

        def out_flush(m, oas, uu, j0, j1):
            nc.sync.dma_start(out=out_d[2 * m + uu, :, j0:j1], in_=oas[uu][:, j0:j1])

        def do_m(m, carry):
            """Column-pipelined stage1 for one m-tile; emits carried stage-2
            window closures from the previous m into PE gaps. Returns this m's
            16 window closures (each also flushes out pieces when complete)."""
            psT = ps_t.tile([128, 512], mybir.dt.float32, name=f"psT{m}", tag="psT")
            ptile = ppool.tile([128, JC], bf, name=f"pt{m}", tag="pt")
            p2s = [p2pool.tile([128, JC], bf, name=f"p2_{m}_{uu}", tag="p2")
                   for uu in range(2)]
            oas = [opool.tile([128, OUT_W], mybir.dt.float16,
                              name=f"oa_{m}_{uu}", tag="o") for uu in range(2)]
            NG = len(GROUPS)
            pend = {}
            carry = list(carry)

            def emit_carry(n=1):
                for _ in range(n):
                    if carry:
                        carry.pop(0)()

            def win_closure(uu, win):
                def f():
                    stage2_win(m, p2s, oas, uu, win)
                    if win == 3:
                        out_flush(m, oas, uu, 0, 2048)
                    elif m == M_TILES - 1 and win >= 4:
                        # final m-tile: flush per 512-col window so the kernel
                        # tail only waits on one small out DMA
                        out_flush(m, oas, uu, 512 * win, 512 * win + 512)
                    elif win == 7:
                        out_flush(m, oas, uu, 2048, OUT_W)
                return f

            # 5 near groups open (5 ps_n banks) while x[0:20] lands, then the
            # whole T chain (u + x tail); V+drain close each group before the
            # next +5 opens
            pend[0] = near_group(m, 0)
            pend[1] = near_group(m, 1)
            emit_carry(1)
            pend[2] = near_group(m, 2)
            emit_carry(1)
            pend[3] = near_group(m, 3)
            emit_carry(1)
            pend[4] = near_group(m, 4)
            t_mms(m, psT, 0, 20)
            emit_carry(1)
            t_mms(m, psT, 20, KH)
            tpair = t_finish(m, psT, 0)
            tt = tpair[1]
            for g in range(NG):
                if g == 3:
                    t_finish(m, tpair, 1)
                ps, c0, c1 = pend.pop(g)
                v_mm(tt, ps, c0, c1)
                nc.vector.tensor_copy(out=ptile[:, c0:c1], in_=ps[:, 0:c1 - c0])
                if g + 5 < NG:
                    pend[g + 5] = near_group(m, g + 5)
                emit_carry(1)
                if g == 4:
                    p2_half(m, ptile, p2s, 0)
                    # this m's first-half windows become available now; queue
                    # them behind carried windows from the previous m
                    carry += [win_closure(uu, w) for w in range(4) for uu in range(2)]
            p2_half(m, ptile, p2s, 1)
            emit_carry(len(carry))
            return [win_closure(uu, w) for w in range(4, 8) for uu in range(2)]

        carry = do_m(0, [])
        load_x(1, nc.sync)
        carry = do_m(1, carry)
        for f in carry:
            f()

    nc.compile()
    return nc


def _prep_inputs(x, w, b):
    blob, dn = _pack_consts(w)
    bias = np.ascontiguousarray(b.reshape(128, 1).astype(np.float32))

    in_maps = []
    for core in range(N_CORES):
        xh = x[core * NPC:(core + 1) * NPC].reshape(NPC * C, W)  # [256, 4096]
        # xeo[m, p, k, r] = xh[128*m + r, 128*k + p]
        xeo = np.ascontiguousarray(
            xh.T.reshape(KH, 128, M_TILES, 128).transpose(2, 1, 0, 3)
        ).astype(BF16)
        in_maps.append({"xeo": xeo, "dn": dn, "cblob": blob, "bias": bias})
    return in_maps


def run(x, w, b, trace=False):
    nc = build_nc()
    in_maps = _prep_inputs(x, w, b)
    try:
        res = run_bass_kernel_spmd(nc, in_maps, list(range(N_CORES)), trace=trace)
    except Exception:
        # transient NRT device errors on first execution after a fresh NEFF
        # load have been observed; one retry has always succeeded
        res = run_bass_kernel_spmd(nc, in_maps, list(range(N_CORES)), trace=trace)
    out = np.empty((N, F, OUT_W), np.float32)
    for core in range(N_CORES):
        out[core * NPC:(core + 1) * NPC] = res.results[core]["out"].astype(np.float32)
    return out, res


def kernel(x, w, b):
    x = np.asarray(x, dtype=np.float32)
    w = np.asarray(w, dtype=np.float32)
    b = np.asarray(b, dtype=np.float32)
    out, _ = run(x, w, b, trace=False)
    return out
